# revision 5
# baseline (speedup 1.0000x reference)
"""Per-pixel depthwise 3x3 conv (Conv2dLocal) on 8 Trainium2 NeuronCores.

out[b,c,h,w] = sum_{i,j in 3x3} x[b,c,h+i-1,w+j-1] * weight[b, c*9+3i+j, h, w]

Sharding: 8 cores = 2 batches x 4 H-slabs of 64 rows (data/spatial parallel).
The host pads the input spatially (1-px halo) and hands every core an
overlapping x slab, so the device program is identical on all cores (pure
SPMD, no collectives).

Memory-bound kernel; per-core fp32 traffic would be ~47 MB against a
~400 GB/s/core DMA-ring cap. The 2e-2 rel-err budget is spent on lossy
weight compression: 4 of 9 taps are stored as int8 (q = round(32*w),
rel err ~0.9%/tap), the remaining 5 as fp16 scaled by 32 (so one uniform
1/32 un-scale on the host output). int8 taps are upcast to fp16 on
ScalarE (1 elem/cycle/lane); fp16 taps multiply straight from DRAM.

Per-core layout: partition p = hb*32 + c (hb: 16-row block, c: channel);
free dim = (row, w), so all nine 3x3 tap shifts are free-dim offsets into
one resident x slab [128, 18, 514] fp16. The j=1 column shift is odd, so
ScalarE maintains a one-element-shifted slab copy (x_od) to keep DVE's
2x packed tensor_tensor mode (needs 4B-aligned, step-1 fp16).

Compute: DVE does the 9 per-tap multiplies in fp16 2x mode; PE
accumulates the products via exact identity-matmul into fp32 PSUM
(start/stop over the taps); ScalarE downcasts PSUM->SBUF fp16 and the
result streams out on the scalar HWDGE ring.

Engine balance (per core): DMA ring ~19.2 MB (~48us), DVE 36 multiplies
(~43us), ScalarE x_od copies + 16 upcasts + 4 output copies (~48us),
PE 144 matmuls (~40us). The weight stream (sync ring) is ordered
[fp16 2-tap chunk, fp16 3-tap chunk, next group's int8 chunk] so the
int8 upcast always has one group of lead time; x and the group-0 int8
chunk ride the scalar ring which is otherwise idle early.
"""

import sys

if "/opt/trn_rl_repo" not in sys.path:
    sys.path.insert(0, "/opt/trn_rl_repo")

from contextlib import ExitStack

import numpy as np

import concourse.mybir as mybir
import concourse.tile as tile
from concourse import bacc
from concourse.bass_utils import run_bass_kernel_spmd
from concourse.masks import make_identity

# Problem shape (hardcoded per harness contract)
B, C, H, W = 2, 32, 256, 512
K = 3
KK = K * K
N_CORES = 8

# Per-core decomposition
HL = H // 4          # 64 local rows per core
HB = 4               # row-blocks per core (partition groups)
RB = HL // HB        # 16 rows per partition
G = 4                # rows processed per group
NGRP = RB // G       # 4 groups
WP = W + 2           # width incl. halo
NP = 128             # partitions

FP32 = mybir.dt.float32
FP16 = mybir.dt.float16
I8 = mybir.dt.int8
MULT = mybir.AluOpType.mult

# Weight tap assignment. SCALE folds into every stored weight; host
# divides the output by SCALE. int8 taps store round(SCALE*w).
SCALE = 32.0
F16_TAPS = [0, 1, 4, 7, 8]   # tap order of the fp16 stream (chunks [0,1], [4,7,8])
I8_TAPS = [3, 6, 2, 5]       # int8 taps (upcast chunks [3,6], [2,5])
# DVE consumption order: fp16 taps first (no upcast dependency), then int8.
TT_ORDER = [0, 1, 4, 7, 8, 3, 6, 2, 5]
NI8 = len(I8_TAPS)

_PROGRAM = None


def _build_program() -> bacc.Bacc:
    nc = bacc.Bacc(
        "TRN2", target_bir_lowering=False, debug=False, num_devices=N_CORES
    )
    x_d = nc.declare_dram_parameter("x", [HB, C, RB + 2, WP], FP16, isOutput=False)
    w_d = nc.declare_dram_parameter(
        "w", [NGRP, HB, C, KK - NI8, G, W], FP16, isOutput=False
    )
    w8_d = nc.declare_dram_parameter(
        "w8", [NGRP, HB, C, NI8, G, W], I8, isOutput=False
    )
    o_d = nc.declare_dram_parameter("o", [NGRP, HB, C, G, W], FP16, isOutput=True)

    with tile.TileContext(nc) as tc, ExitStack() as ctx:
        x_pool = ctx.enter_context(tc.tile_pool(name="x", bufs=1))
        xo_pool = ctx.enter_context(tc.tile_pool(name="xod", bufs=1))
        wa_pool = ctx.enter_context(tc.tile_pool(name="wa", bufs=3))
        wb_pool = ctx.enter_context(tc.tile_pool(name="wb", bufs=3))
        w8_pool = ctx.enter_context(tc.tile_pool(name="wt8", bufs=3))
        w8f_pool = ctx.enter_context(tc.tile_pool(name="wt8f", bufs=3))
        prod_pool = ctx.enter_context(tc.tile_pool(name="prod", bufs=6))
        out_pool = ctx.enter_context(tc.tile_pool(name="outsb", bufs=2))
        const_pool = ctx.enter_context(tc.tile_pool(name="const", bufs=1))
        pe_pool = ctx.enter_context(tc.tile_pool(name="pe", bufs=2, space="PSUM"))

        ident = const_pool.tile([NP, NP], FP16)
        make_identity(nc, ident)

        # x slab: per partition 18 rows (16 + 2 halo) x 514 cols. Rows 0:6
        # (all of group 0's inputs) land first; the scalar ring is idle
        # early so x and group 0's int8 chunk ride it.
        x_sb = x_pool.tile([NP, RB + 2, WP], FP16)
        nc.scalar.dma_start(out=x_sb[:, 0:6, :], in_=x_d[:, :, 0:6, :])

        def fetch_w8(g, engine):
            wt8 = w8_pool.tile([NP, NI8, G, W], I8, tag="wt8", name=f"wt8_{g}")
            engine.dma_start(out=wt8, in_=w8_d[g])
            return wt8

        w8_tiles = [None] * NGRP
        w8_tiles[0] = fetch_w8(0, nc.scalar)
        nc.scalar.dma_start(
            out=x_sb[:, 6 : RB + 2, :], in_=x_d[:, :, 6 : RB + 2, :]
        )

        # Odd-shifted slab copy (cols 1..512) so j=1 taps read 4B-aligned.
        # Split so group 0's rows are ready early; interleaved with the
        # group-0 upcasts on ScalarE.
        x_od = xo_pool.tile([NP, RB + 2, W], FP16)
        w8f = [None] * NGRP

        def upcast(g, lo, hi):
            if w8f[g] is None:
                w8f[g] = w8f_pool.tile(
                    [NP, NI8, G, W], FP16, tag="wt8f", name=f"w8f_{g}"
                )
            nc.scalar.copy(out=w8f[g][:, lo:hi], in_=w8_tiles[g][:, lo:hi])

        nc.scalar.copy(out=x_od[:, 0:6, :], in_=x_sb[:, 0:6, 1 : 1 + W])
        upcast(0, 0, 1)
        upcast(0, 1, 2)
        nc.scalar.copy(out=x_od[:, 6:12, :], in_=x_sb[:, 6:12, 1 : 1 + W])
        upcast(0, 2, 3)
        upcast(0, 3, 4)
        nc.scalar.copy(out=x_od[:, 12 : RB + 2, :], in_=x_sb[:, 12 : RB + 2, 1 : 1 + W])

        for grp in range(NGRP):
            R = grp * G
            # fp16 weight chunks for this group on the sync ring, then the
            # NEXT group's int8 chunk (one group of upcast lead time).
            wa = wa_pool.tile([NP, 2, G, W], FP16, tag="wa")
            nc.sync.dma_start(out=wa, in_=w_d[grp, :, :, 0:2])
            wb = wb_pool.tile([NP, 3, G, W], FP16, tag="wb")
            nc.sync.dma_start(out=wb, in_=w_d[grp, :, :, 2:5])
            if grp + 1 < NGRP:
                w8_tiles[grp + 1] = fetch_w8(grp + 1, nc.sync)

            acc = pe_pool.tile([NP, G, W], FP32, tag="acc")
            for idx, t in enumerate(TT_ORDER):
                i, j = t // K, t % K
                fpos = F16_TAPS.index(t) if t in F16_TAPS else -1
                if fpos == -1:
                    wt = w8f[grp][:, I8_TAPS.index(t)]
                elif fpos < 2:
                    wt = wa[:, fpos]
                else:
                    wt = wb[:, fpos - 2]
                if j == 1:
                    xin = x_od[:, R + i : R + i + G, :]
                else:
                    xin = x_sb[:, R + i : R + i + G, j : j + W]
                prod = prod_pool.tile([NP, G, W], FP16, tag="prod")
                nc.vector.tensor_tensor(prod[:], wt, xin, MULT)
                # Exact accumulation: ident.T @ prod == prod, summed into
                # fp32 PSUM across the taps (one matmul per PSUM bank).
                for c in range(G):
                    nc.tensor.matmul(
                        acc[:, c, :],
                        ident[:],
                        prod[:, c, :],
                        start=(idx == 0),
                        stop=(idx == KK - 1),
                        skip_group_check=True,
                    )
            # upcast the next group's int8 chunk between this group's
            # compute and output drain
            if grp + 1 < NGRP:
                upcast(grp + 1, 0, 2)
                upcast(grp + 1, 2, 4)
            out_sb = out_pool.tile([NP, G, W], FP16, tag="outsb")
            if grp == NGRP - 1:
                # Pipeline the drain: each half's copy starts as soon as
                # its PSUM banks hit their stop-matmul, and the first
                # half's store overlaps the second half's copy.
                h = G // 2
                nc.scalar.copy(out=out_sb[:, 0:h, :], in_=acc[:, 0:h, :])
                nc.scalar.dma_start(
                    out=o_d[grp, :, :, 0:h, :], in_=out_sb[:, 0:h, :]
                )
                nc.scalar.copy(out=out_sb[:, h:G, :], in_=acc[:, h:G, :])
                nc.scalar.dma_start(
                    out=o_d[grp, :, :, h:G, :], in_=out_sb[:, h:G, :]
                )
            else:
                nc.scalar.copy(out=out_sb[:], in_=acc[:])
                nc.scalar.dma_start(out=o_d[grp], in_=out_sb[:])

    nc.compile()
    return nc


def _get_program() -> bacc.Bacc:
    global _PROGRAM
    if _PROGRAM is None:
        _PROGRAM = _build_program()
    return _PROGRAM


def _shard_inputs(input: np.ndarray, weight: np.ndarray) -> list[dict]:
    xp = np.pad(input, ((0, 0), (0, 0), (1, 1), (1, 1))).astype(np.float16)
    ws = weight.astype(np.float32) * SCALE
    in_maps = []
    for k in range(N_CORES):
        b, hb = k // 4, k % 4
        h0 = hb * HL
        xs = xp[b, :, h0 : h0 + HL + 2, :]  # [C, 66, WP]
        # x: the HB overlapping 18-row windows -> [HB, C, 18, WP]
        x4 = np.ascontiguousarray(
            np.stack([xs[:, r0 : r0 + RB + 2, :] for r0 in range(0, HL, RB)])
        )
        # weights -> [grp, tap, hb, c, r, w], taps pre-ordered per
        # F16_TAPS / I8_TAPS so each device DMA reads one linear block.
        w6 = (
            ws[b]
            .reshape(C, KK, H, W)[:, :, h0 : h0 + HL, :]
            .reshape(C, KK, HB, NGRP, G, W)
            .transpose(3, 1, 2, 0, 4, 5)
        )  # [grp, tap, hb, c, r, w]
        # partition dims (hb, c) outermost, tap dim inside, so multi-tap
        # DMA chunks iterate in the same element order as the SBUF tile
        wf = np.ascontiguousarray(
            w6[:, F16_TAPS].transpose(0, 2, 3, 1, 4, 5)
        ).astype(np.float16)
        w8 = np.clip(
            np.rint(w6[:, I8_TAPS].transpose(0, 2, 3, 1, 4, 5)), -127, 127
        ).astype(np.int8)
        in_maps.append({"x": x4, "w": wf, "w8": w8})
    return in_maps


def kernel(input: np.ndarray, weight: np.ndarray, _trace: bool = False):
    nc = _get_program()
    in_maps = _shard_inputs(np.asarray(input), np.asarray(weight))
    res = run_bass_kernel_spmd(
        nc, in_maps, core_ids=list(range(N_CORES)), trace=_trace
    )
    out = np.empty((B, C, H, W), dtype=np.float32)
    inv = 1.0 / SCALE
    for k in range(N_CORES):
        b, hb = k // 4, k % 4
        # device out [grp, hb, c, r, w] -> [c, hb*16 + grp*4 + r, w]
        o = (
            res.results[k]["o"]
            .reshape(NGRP, HB, C, G, W)
            .transpose(2, 1, 0, 3, 4)
            .reshape(C, HL, W)
            .astype(np.float32)
        )
        out[b, :, hb * HL : (hb + 1) * HL, :] = o * inv
    if _trace:
        return out, res
    return out
